# revision 20
# baseline (speedup 1.0000x reference)
"""Trainium2 Bass kernel for ByteSeq2Seq (2-layer LSTM encoder + greedy LSTM decoder).

Strategy: data-parallel over batch (B=128 -> 16 per core x 8 cores, no
collectives).  All recurrent matmuls keep the (transposed) hidden state as the
stationary operand and stream the weights as the moving operand, so the
per-step tensor-engine cost is weight-columns / 2.4GHz regardless of the local
batch size.  Layer-0 input projections are folded into per-token tables
(emb @ W_ih.T + biases, 256 x 2048) applied via one-hot matmuls.  The two
encoder layers run interleaved (layer 1 lags one chunk) so each layer's
latency chain hides under the other's engine work.

Gate order is permuted to [i, f, o, g] so sigmoid gates are contiguous.
Gate PSUM is processed in 512-column chunks (1 bank each, 2 bufs/layer) to fit
all concurrent users in the 8 PSUM banks.
"""

import numpy as np

P = 128          # partitions
BL = 16          # local batch per core
NCORES = 8
V = 256          # vocab
E = 128          # embed dim
H = 512          # hidden
G = 4 * H        # gates = 2048
S = 1024         # src length
T = 256          # decode steps
KH = H // P      # h-chunks (4)
NG = G // 512    # 512-col gate chunks (4)

# new gate col ranges: i=[0:512) f=[512:1024) o=[1024:1536) g=[1536:2048)
N_I, N_F, N_O, N_G = 0, 1, 2, 3
N_ORDER = (N_F, N_I, N_G, N_O)   # emission order: f first (c-update chain)


def _gate_perm():
    a = np.arange(G)
    return np.concatenate([a[0:512], a[512:1024], a[1536:2048], a[1024:1536]])


def _pack_wT(w, perm):
    """w: [G, K] torch-layout -> packed moving operand [128, (K/128)*G]."""
    wt = w[perm].T  # [K, G]
    k = wt.shape[0] // P
    return np.ascontiguousarray(
        wt.reshape(k, P, G).transpose(1, 0, 2).reshape(P, k * G)
    ).astype(np.float32)


def _host_prep(inp):
    perm = _gate_perm()

    def tbl(emb, w_ih, b_ih, b_hh):
        t = (emb @ w_ih.T + b_ih + b_hh)[:, perm]        # [V, G]
        return np.ascontiguousarray(
            t.reshape(2, P, G).transpose(1, 0, 2).reshape(P, 2 * G)
        ).astype(np.float32)

    shared = dict(
        tbl_e=tbl(inp["enc_emb"], inp["enc_w_ih0"], inp["enc_b_ih0"], inp["enc_b_hh0"]),
        tbl_d=tbl(inp["dec_emb"], inp["dec_w_ih0"], inp["dec_b_ih0"], inp["dec_b_hh0"]),
        w_hh0e=_pack_wT(inp["enc_w_hh0"], perm),
        w_ih1e=_pack_wT(inp["enc_w_ih1"], perm),
        w_hh1e=_pack_wT(inp["enc_w_hh1"], perm),
        w_hh0d=_pack_wT(inp["dec_w_hh0"], perm),
        w1d=np.concatenate([_pack_wT(inp["dec_w_ih1"], perm),
                            _pack_wT(inp["dec_w_hh1"], perm)], axis=1),
        fcw=np.ascontiguousarray(
            inp["fc_w"].T.reshape(KH, P, V).transpose(1, 0, 2).reshape(P, KH * V)
        ).astype(np.float32),
        b1e=(inp["enc_b_ih1"] + inp["enc_b_hh1"])[perm].reshape(1, G).astype(np.float32),
        b1d=(inp["dec_b_ih1"] + inp["dec_b_hh1"])[perm].reshape(1, G).astype(np.float32),
        fcb=inp["fc_b"].reshape(1, V).astype(np.float32),
        iota2=np.stack([np.arange(P), np.arange(P) + P], axis=1).astype(np.float32),
        id128=np.eye(P, dtype=np.float32),
    )

    src = np.asarray(inp["src"])
    trg = np.asarray(inp["trg"])
    maps = []
    for c in range(NCORES):
        sl = slice(c * BL, (c + 1) * BL)
        m = dict(shared)
        m["src_row"] = np.ascontiguousarray(
            src[sl].T.reshape(1, -1)).astype(np.float32)    # time-major [1, S*16]
        m["tok0f"] = trg[sl, 0:1].astype(np.float32)        # [16, 1]
        maps.append(m)
    return maps



def _split_matmul_waits(nc, mybir):
    """fp32 Matmult (implicit LDWEIGHTS) can only encode one sync wait in
    walrus codegen; move matmul waits onto a preceding PE EventSemaphore."""
    f = nc.m.functions[0]
    for bb in f.blocks:
        insts = bb.instructions
        idx = 0
        while idx < len(insts):
            inst = insts[idx]
            if isinstance(inst, mybir.InstMatmult):
                si = inst.sync_info
                w = list(si.on_wait)
                if len(w) > 1:
                    es = mybir.InstEventSemaphore(
                        name=nc.get_next_instruction_name())
                    es.engine = mybir.EngineType.PE
                    nc.register_instruction(es)
                    es.sync_info = mybir.SyncInfo(on_wait=w, on_update=[])
                    si.on_wait = []
                    insts.insert(idx, es)
                    idx += 1
            idx += 1


_CACHED_NC = None


def _build_nc(s_len=None, t_len=None, chunk=8, part="all"):
    s_len = S if s_len is None else s_len
    t_len = T if t_len is None else t_len
    import concourse.bacc as bacc
    import concourse.mybir as mybir
    import concourse.tile as tile
    from contextlib import ExitStack

    F32 = mybir.dt.float32
    U32 = mybir.dt.uint32
    AF = mybir.ActivationFunctionType
    ALU = mybir.AluOpType

    nc = bacc.Bacc("TRN2", target_bir_lowering=False, debug=False,
                   num_devices=NCORES)

    def din(name, shape):
        return nc.dram_tensor(name, shape, F32, kind="ExternalInput").ap()

    src_row = din("src_row", (1, s_len * BL))
    tok0f = din("tok0f", (BL, 1))
    tbl_e = din("tbl_e", (P, 2 * G))
    tbl_d = din("tbl_d", (P, 2 * G))
    w_hh0e = din("w_hh0e", (P, KH * G))
    w_ih1e = din("w_ih1e", (P, KH * G))
    w_hh1e = din("w_hh1e", (P, KH * G))
    w_hh0d = din("w_hh0d", (P, KH * G))
    w1d = din("w1d", (P, 2 * KH * G))
    fcw = din("fcw", (P, KH * V))
    b1e = din("b1e", (1, G))
    b1d = din("b1d", (1, G))
    fcb = din("fcb", (1, V))
    iota2 = din("iota2", (P, 2))
    id128 = din("id128", (P, P))
    out_log = nc.dram_tensor("logits", (BL, t_len * V), F32,
                             kind="ExternalOutput").ap()

    n_chunks = s_len // chunk
    CHT = chunk * BL  # tokens per chunk

    with tile.TileContext(nc) as tc, ExitStack() as ctx:
        constp = ctx.enter_context(tc.tile_pool(name="const", bufs=1))
        statep = ctx.enter_context(tc.tile_pool(name="state", bufs=1))
        dramp = ctx.enter_context(tc.tile_pool(name="dram", bufs=2, space="DRAM"))

        id_sb = constp.tile([BL, BL], F32)
        nc.sync.dma_start(id_sb[:], id128[:BL, :BL])
        iota_sb = constp.tile([P, 2], F32)
        nc.sync.dma_start(iota_sb[:], iota2)
        ones_sb = constp.tile([1, P], F32)
        nc.vector.memset(ones_sb[:], 1.0)
        z64 = constp.tile([P, 64], F32)
        nc.vector.memset(z64[:], 0.0)

        c0 = statep.tile([BL, H], F32)
        c1 = statep.tile([BL, H], F32)
        nc.vector.memset(c0[:], 0.0)
        nc.vector.memset(c1[:], 0.0)
        h0T_d = statep.tile([P, 64], F32)   # decoder-init / decoder h0T
        h1T_d = statep.tile([P, 64], F32)

        def gate_chunk_math(n, gsrc, sig, tg, pre=None, xg=None):
            """per-n-chunk gate math: optional xg add (psum+sbuf->sbuf), then
            sigmoid/tanh."""
            if xg is not None:
                nc.vector.tensor_add(pre[:], gsrc,
                                     xg[:, n * 512:(n + 1) * 512])
                gsrc = pre[:]
            if n == N_G:
                nc.scalar.activation(tg[:], gsrc, AF.Tanh)
            else:
                nc.scalar.activation(sig[:, n * 512:(n + 1) * 512], gsrc,
                                     AF.Sigmoid)

        def cell_update(sig, tg, cstate, t1, h):
            nc.vector.tensor_mul(t1[:], sig[:, N_F * 512:(N_F + 1) * 512], cstate[:])
            nc.vector.tensor_mul(tg[:], sig[:, N_I * 512:(N_I + 1) * 512], tg[:])
            nc.vector.tensor_add(cstate[:], t1[:], tg[:])
            nc.scalar.activation(t1[:], cstate[:], AF.Tanh)
            nc.vector.tensor_mul(h[:], sig[:, N_O * 512:(N_O + 1) * 512], t1[:])

        def transpose_h(h, pspool, tag, dst_ap4):
            trp = pspool.tile([P, KH * BL], F32, tag=tag, space="PSUM")
            for k in range(KH):
                nc.tensor.transpose(trp[:, k * BL:(k + 1) * BL],
                                    h[:, k * P:(k + 1) * P],
                                    id_sb[:BL, :BL])
            nc.vector.tensor_copy(
                dst_ap4, trp[:].rearrange("p (k b) -> p k b", k=KH))

        # ---------------- encoder ----------------
        with tc.tile_pool(name="encw", bufs=1) as encw, \
             tc.tile_pool(name="enc_s", bufs=2) as sp, \
             tc.tile_pool(name="enc_t", bufs=1) as tmp, \
             tc.tile_pool(name="y0t", bufs=2) as y0tp, \
             tc.tile_pool(name="ps_g0", bufs=2, space="PSUM") as ps_g0, \
             tc.tile_pool(name="ps_g1", bufs=2, space="PSUM") as ps_g1, \
             tc.tile_pool(name="ps_tr", bufs=1, space="PSUM") as ps_tr, \
             tc.tile_pool(name="ps_aux", bufs=2, space="PSUM") as ps_aux:

            tbl_e_sb = encw.tile([P, 2 * G], F32)
            nc.sync.dma_start(tbl_e_sb[:], tbl_e)
            whh0 = encw.tile([P, KH * G], F32)
            nc.sync.dma_start(whh0[:], w_hh0e)
            whh1 = encw.tile([P, KH * G], F32)
            nc.sync.dma_start(whh1[:], w_hh1e)
            wih1 = encw.tile([P, KH * G], F32)
            nc.sync.dma_start(wih1[:], w_ih1e)
            b1e_sb = encw.tile([1, G], F32)
            nc.sync.dma_start(b1e_sb[:], b1e)
            tc.strict_bb_all_engine_barrier()

            y0T = [y0tp.tile([P, KH * CHT], F32, tag="y0T", name=f"y0T{i}")
                   for i in range(2)]
            xg1_dram = None
            if part != "l0":
                xg1_dram = [dramp.tile([CHT, G], F32, tag="xg1d", name=f"xg1d{i}")
                            for i in range(2)]

            h1T_prev = statep.tile([P, 2 * 64], F32)
            nc.vector.memset(h1T_prev[:], 0.0)

            state = {}

            def l0_step(t):
                cidx = t // chunk
                par = cidx % 2
                tloc = t % chunk
                g4 = t % 4
                if g4 == 0:
                    # tokens for steps t..t+4, padded to 32-partition stride
                    srow = sp.tile([1, P], F32, tag="srow")
                    nc.vector.memset(srow[:], 0.0)
                    nc.sync.dma_start(
                        srow[:].rearrange("o (q r) -> o q r", r=32)[:, :, :BL],
                        src_row[:, t * BL:(t + 4) * BL].rearrange(
                            "o (q r) -> o q r", r=BL))
                    repp = ps_aux.tile([P, P], F32, tag="aux", space="PSUM")
                    nc.tensor.matmul(repp[:], ones_sb[:], srow[:],
                                     start=True, stop=True)
                    oh = sp.tile([P, 2 * P], F32, tag="oh")
                    for cc in range(2):
                        nc.vector.tensor_tensor(
                            out=oh[:, cc * P:(cc + 1) * P],
                            in0=iota_sb[:, cc:cc + 1].to_broadcast([P, P]),
                            in1=repp[:],
                            op=ALU.is_equal)
                    xg0 = sp.tile([P, G], F32, tag="xg0")
                    for n in range(NG):
                        xp = ps_aux.tile([P, 512], F32, tag="aux", space="PSUM")
                        for cc in range(2):
                            nc.tensor.matmul(
                                xp[:],
                                oh[:, cc * P:(cc + 1) * P],
                                tbl_e_sb[:, cc * G + n * 512: cc * G + (n + 1) * 512],
                                start=(cc == 0), stop=(cc == 1))
                        nc.vector.tensor_copy(xg0[:, n * 512:(n + 1) * 512], xp[:])
                    state["xg0"] = xg0
                if t == 0:
                    lhsT = [z64[:, k * BL:(k + 1) * BL] for k in range(KH)]
                else:
                    tp, pp = (tloc - 1, par) if tloc > 0 else (chunk - 1, 1 - par)
                    y3 = y0T[pp][:].rearrange("p (k c) -> p k c", k=KH)
                    lhsT = [y3[:, k, tp * BL:(tp + 1) * BL] for k in range(KH)]
                sig = tmp.tile([BL, 3 * H], F32, tag="sig0")
                tg = tmp.tile([BL, H], F32, tag="tg0")
                t1 = tmp.tile([BL, H], F32, tag="t10")
                h = sp.tile([BL, H], F32, tag="h0")
                xg = state["xg0"][g4 * 32:g4 * 32 + BL, :]
                for n in N_ORDER:
                    gp = ps_g0.tile([BL, 512], F32, tag="g0", space="PSUM")
                    for k in range(KH):
                        nc.tensor.matmul(
                            gp[:], lhsT[k],
                            whh0[:, k * G + n * 512: k * G + (n + 1) * 512],
                            start=(k == 0), stop=(k == KH - 1))
                    pre = sp.tile([BL, 512], F32, tag="pre0")
                    gate_chunk_math(n, gp[:], sig, tg, pre=pre, xg=xg)
                cell_update(sig, tg, c0, t1, h)
                y3 = y0T[par][:].rearrange("p (k c) -> p k c", k=KH)
                transpose_h(h, ps_tr, "tr0", y3[:, :, tloc * BL:(tloc + 1) * BL])
                if t == s_len - 1:
                    nc.vector.tensor_copy(
                        h0T_d[:].rearrange("p (k b) -> p k b", k=KH),
                        y3[:, :, tloc * BL:(tloc + 1) * BL])

            def xg1_burst(cidx):
                par = cidx % 2
                y3 = y0T[par][:].rearrange("p (k c) -> p k c", k=KH)
                for grp in range(CHT // P):
                    for nh in range(2):
                        xo = sp.tile([P, G // 2], F32, tag="xg1o", bufs=1)
                        for n2 in range(2):
                            n = nh * 2 + n2
                            xp = ps_aux.tile([P, 512], F32, tag="aux", space="PSUM")
                            nc.tensor.matmul(xp[:], ones_sb[:],
                                             b1e_sb[:, n * 512:(n + 1) * 512],
                                             start=True, stop=False)
                            for k in range(KH):
                                nc.tensor.matmul(
                                    xp[:],
                                    y3[:, k, grp * P:(grp + 1) * P],
                                    wih1[:, k * G + n * 512: k * G + (n + 1) * 512],
                                    start=False, stop=(k == KH - 1))
                            nc.vector.tensor_copy(xo[:, n2 * 512:(n2 + 1) * 512], xp[:])
                        nc.sync.dma_start(
                            xg1_dram[par][grp * P:(grp + 1) * P,
                                          nh * (G // 2):(nh + 1) * (G // 2)],
                            xo[:])

            def l1_step(t):
                cidx = t // chunk
                par = cidx % 2
                g4 = t % 4
                pp = t % 2
                if g4 == 0:
                    xg1_sb = sp.tile([P, G], F32, tag="xg1in")
                    for q in range(4):
                        r0 = ((t % chunk) + q) * BL
                        nc.sync.dma_start(
                            xg1_sb[q * 32:q * 32 + BL, :],
                            xg1_dram[par][r0:r0 + BL, :])
                    state["xg1"] = xg1_sb
                if t == 0:
                    lhsT = [z64[:, k * BL:(k + 1) * BL] for k in range(KH)]
                else:
                    prev = h1T_prev[:].rearrange("p (q k b) -> p q k b", q=2, k=KH)
                    lhsT = [prev[:, 1 - pp, k, :] for k in range(KH)]
                sig = tmp.tile([BL, 3 * H], F32, tag="sig1")
                tg = tmp.tile([BL, H], F32, tag="tg1")
                t1 = tmp.tile([BL, H], F32, tag="t11")
                h = sp.tile([BL, H], F32, tag="h1")
                xg = state["xg1"][g4 * 32:g4 * 32 + BL, :]
                for n in N_ORDER:
                    gp = ps_g1.tile([BL, 512], F32, tag="g1", space="PSUM")
                    for k in range(KH):
                        nc.tensor.matmul(
                            gp[:], lhsT[k],
                            whh1[:, k * G + n * 512: k * G + (n + 1) * 512],
                            start=(k == 0), stop=(k == KH - 1))
                    pre = sp.tile([BL, 512], F32, tag="pre1")
                    gate_chunk_math(n, gp[:], sig, tg, pre=pre, xg=xg)
                cell_update(sig, tg, c1, t1, h)
                prev = h1T_prev[:].rearrange("p (q k b) -> p q k b", q=2, k=KH)
                transpose_h(h, ps_tr, "tr1", prev[:, pp, :, :])
                if t == s_len - 1:
                    nc.vector.tensor_copy(
                        h1T_d[:].rearrange("p (k b) -> p k b", k=KH),
                        prev[:, pp, :, :])

            if part in ("all", "enc", "l0", "l0x"):
                for tt in range(s_len + chunk):
                    if tt < s_len:
                        l0_step(tt)
                        if part != "l0" and tt % chunk == chunk - 1:
                            xg1_burst(tt // chunk)
                    if part in ("all", "enc") and tt >= chunk:
                        l1_step(tt - chunk)
            else:
                nc.vector.memset(h0T_d[:], 0.0)
                nc.vector.memset(h1T_d[:], 0.0)
            if part in ("l0", "l0x"):
                nc.vector.memset(h1T_d[:], 0.0)
            if part in ("enc", "l0", "l0x"):
                nc.sync.dma_start(out_log[:, 0:H], c0[:])
                nc.sync.dma_start(out_log[:, H:2 * H], c1[:])
                for q in range(4):
                    nc.sync.dma_start(
                        out_log[:BL, (2 * H + q * 64):(2 * H + (q + 1) * 64)],
                        h0T_d[q * 32:q * 32 + BL, :])
                    nc.sync.dma_start(
                        out_log[:BL, (2 * H + 256 + q * 64):(2 * H + 256 + (q + 1) * 64)],
                        h1T_d[q * 32:q * 32 + BL, :])

        # ---------------- decoder ----------------
        if part in ("enc", "l0", "l0x"):
            run_decoder = False
        else:
            run_decoder = True
        if run_decoder:
         with tc.tile_pool(name="decw", bufs=1) as decw, \
             tc.tile_pool(name="dec_s", bufs=2) as dp, \
             tc.tile_pool(name="dec_t", bufs=1) as dtmp, \
             tc.tile_pool(name="ps_dg", bufs=2, space="PSUM") as ps_dg, \
             tc.tile_pool(name="ps_dtr", bufs=1, space="PSUM") as ps_dtr, \
             tc.tile_pool(name="ps_daux", bufs=2, space="PSUM") as ps_daux:

            tbl_d_sb = decw.tile([P, 2 * G], F32)
            nc.sync.dma_start(tbl_d_sb[:], tbl_d)
            whh0d = decw.tile([P, KH * G], F32)
            nc.sync.dma_start(whh0d[:], w_hh0d)
            w1d_sb = decw.tile([P, 2 * KH * G], F32)
            nc.sync.dma_start(w1d_sb[:], w1d)
            fcw_sb = decw.tile([P, KH * V], F32)
            nc.sync.dma_start(fcw_sb[:], fcw)
            b1d_sb = decw.tile([1, G], F32)
            nc.sync.dma_start(b1d_sb[:], b1d)
            fcb_sb = decw.tile([1, V], F32)
            nc.sync.dma_start(fcb_sb[:], fcb)
            tc.strict_bb_all_engine_barrier()

            idxf = statep.tile([BL, 1], F32)
            nc.sync.dma_start(idxf[:], tok0f)

            h0T = h0T_d[:].rearrange("p (k b) -> p k b", k=KH)
            h1T = h1T_d[:].rearrange("p (k b) -> p k b", k=KH)

            for t in range(t_len):
                # token -> transposed one-hot [128, 2*BL]
                itp = ps_daux.tile([1, BL], F32, tag="daux", space="PSUM")
                nc.tensor.transpose(itp[:], idxf[:], id_sb[:BL, :BL])
                itr = dp.tile([1, BL], F32, tag="itr")
                nc.vector.tensor_copy(itr[:], itp[:])
                repp = ps_daux.tile([P, BL], F32, tag="daux", space="PSUM")
                nc.tensor.matmul(repp[:], ones_sb[:], itr[:],
                                 start=True, stop=True)
                oh = dp.tile([P, 2 * BL], F32, tag="doh")
                for cc in range(2):
                    nc.vector.tensor_tensor(
                        out=oh[:, cc * BL:(cc + 1) * BL],
                        in0=iota_sb[:, cc:cc + 1].to_broadcast([P, BL]),
                        in1=repp[:],
                        op=ALU.is_equal)
                # ---- cell 0  (gates = tbl_d[tok] + h0 @ Whh0d.T)
                sig = dtmp.tile([BL, 3 * H], F32, tag="dsig")
                tg = dtmp.tile([BL, H], F32, tag="dtg")
                t1 = dtmp.tile([BL, H], F32, tag="dt1")
                h0 = dp.tile([BL, H], F32, tag="dh0")
                for n in N_ORDER:
                    gp = ps_dg.tile([BL, 512], F32, tag="dg", space="PSUM")
                    for k in range(KH):
                        nc.tensor.matmul(
                            gp[:], h0T[:, k, :],
                            whh0d[:, k * G + n * 512: k * G + (n + 1) * 512],
                            start=(k == 0), stop=False)
                    for cc in range(2):
                        nc.tensor.matmul(
                            gp[:], oh[:, cc * BL:(cc + 1) * BL],
                            tbl_d_sb[:, cc * G + n * 512: cc * G + (n + 1) * 512],
                            start=False, stop=(cc == 1))
                    gate_chunk_math(n, gp[:], sig, tg)
                cell_update(sig, tg, c0, t1, h0)
                transpose_h(h0, ps_dtr, "dtr", h0T)
                # ---- cell 1  (gates = b1 + h1 @ Whh1.T + h0 @ Wih1.T)
                sig1 = dtmp.tile([BL, 3 * H], F32, tag="dsig1")
                tg1 = dtmp.tile([BL, H], F32, tag="dtg1")
                t11 = dtmp.tile([BL, H], F32, tag="dt11")
                h1 = dp.tile([BL, H], F32, tag="dh1")
                for n in N_ORDER:
                    gp = ps_dg.tile([BL, 512], F32, tag="dg", space="PSUM")
                    nc.tensor.matmul(gp[:], ones_sb[:, :BL],
                                     b1d_sb[:, n * 512:(n + 1) * 512],
                                     start=True, stop=False)
                    for k in range(KH):
                        nc.tensor.matmul(
                            gp[:], h1T[:, k, :],
                            w1d_sb[:, (KH + k) * G + n * 512:
                                   (KH + k) * G + (n + 1) * 512],
                            start=False, stop=False)
                    for k in range(KH):
                        nc.tensor.matmul(
                            gp[:], h0T[:, k, :],
                            w1d_sb[:, k * G + n * 512: k * G + (n + 1) * 512],
                            start=False, stop=(k == KH - 1))
                    gate_chunk_math(n, gp[:], sig1, tg1)
                cell_update(sig1, tg1, c1, t11, h1)
                transpose_h(h1, ps_dtr, "dtr", h1T)
                # ---- fc + argmax
                lp = ps_daux.tile([BL, V], F32, tag="daux", space="PSUM")
                nc.tensor.matmul(lp[:], ones_sb[:, :BL], fcb_sb[:],
                                 start=True, stop=False)
                for k in range(KH):
                    nc.tensor.matmul(lp[:], h1T[:, k, :],
                                     fcw_sb[:, k * V:(k + 1) * V],
                                     start=False, stop=(k == KH - 1))
                logit = dp.tile([BL, V], F32, tag="logit")
                nc.vector.tensor_copy(logit[:], lp[:])
                nc.sync.dma_start(out_log[:, t * V:(t + 1) * V], logit[:])
                if t < t_len - 1:
                    vmax = dp.tile([BL, 8], F32, tag="vmax")
                    vidx = dp.tile([BL, 8], U32, tag="vidx")
                    nc.vector.max(vmax[:], logit[:])
                    nc.vector.max_index(vidx[:], vmax[:], logit[:])
                    nc.vector.tensor_copy(idxf[:], vidx[:, 0:1])

    _split_matmul_waits(nc, mybir)
    nc.compile()
    return nc


def _get_nc():
    global _CACHED_NC
    if _CACHED_NC is None:
        _CACHED_NC = _build_nc()
    return _CACHED_NC


def kernel(**inputs):
    from concourse import bass_utils
    in_maps = _host_prep(inputs)
    nc = _get_nc()
    res = bass_utils.run_bass_kernel_spmd(nc, in_maps, core_ids=list(range(NCORES)))
    outs = [r["logits"].reshape(BL, T, V) for r in res.results]
    return np.concatenate(outs, axis=0)
